# revision 3
# baseline (speedup 1.0000x reference)
"""Supervised-contrastive loss on 8 Trainium2 NeuronCores.

Math (reference):
    z = x / max(||x||, 1e-8)                  row-normalize
    sim = (z @ z.T) / TEMP                    [N, N]
    per-anchor: pos-mean over same-class (excl. self) and logsumexp over
    j != i, then per-class mean, then mean over classes.

Distribution: rows of z (anchors) are sharded 1024/core across 8 cores;
z is replicated.  Each core computes its [1024, 8192] slab of raw cosine
similarities (bf16 inputs, fp32 accumulate) and produces two tiny
reductions of it:
    es[i]    = sum_j exp(10 * sim_raw[i, j])          (incl. diagonal)
    tm[i, c] = sum_{j in class c} sim_raw[i, j]       (incl. diagonal)
The class-segment sums are folded into the GEMM: tm = A @ W.T where
W[c] = sum of z rows of class c (precomputed on host), so no masking is
needed on device.  The diagonal term sim_raw[i,i] = ||z_bf16[i]||^2 is
reconstructed exactly on host and subtracted there.  The final
(8192-element) logsumexp/segment-mean arithmetic is negligible host work.
"""

import numpy as np
import ml_dtypes

N = 8192          # anchors
D = 768           # feature dim
NOP = 64          # number of classes
CORES = 8
SLAB = N // CORES  # 1024 anchors per core
KT = D // 128      # 6 contraction tiles
MT = SLAB // 128   # 8 anchor chunks of 128 (PSUM partition dim)
JW = 512           # similarity tile width (one PSUM bank)
GW = 2048          # j-group width (DMA chunk of z^T)
NG = N // GW       # 4 groups
GJ = GW // JW      # 4 tiles per group
NJ = N // JW       # 16 j tiles
TEMP_INV = 10.0
EPS = 1e-8

BF16 = ml_dtypes.bfloat16

_CACHE = {}
LAST_RESULT = None  # BassKernelResults of the most recent run (for profiling)


def _build_nc():
    from concourse import bacc
    import concourse.mybir as mybir
    import concourse.tile as tile

    bf16 = mybir.dt.bfloat16
    f32 = mybir.dt.float32
    Exp = mybir.ActivationFunctionType.Exp

    nc = bacc.Bacc(
        "TRN2", target_bir_lowering=False, debug=False, enable_asserts=False
    )
    zt = nc.dram_tensor("zt", [D, N], bf16, kind="ExternalInput").ap()
    at = nc.dram_tensor("at", [D, SLAB], bf16, kind="ExternalInput").ap()
    wt = nc.dram_tensor("wt", [D, NOP], bf16, kind="ExternalInput").ap()
    es = nc.dram_tensor("es", [128, MT], f32, kind="ExternalOutput").ap()
    tm = nc.dram_tensor("tm", [128, MT, NOP], f32, kind="ExternalOutput").ap()

    zt_r = zt.rearrange("(k p) n -> k p n", p=128)   # [KT, 128, N]
    at_r = at.rearrange("(k p) n -> k p n", p=128)   # [KT, 128, SLAB]
    wt_r = wt.rearrange("(k p) c -> p k c", p=128)   # [128, KT, NOP]

    with tile.TileContext(nc) as tc:
        with (
            tc.tile_pool(name="zin", bufs=KT * NG) as zin,
            tc.tile_pool(name="ain", bufs=KT) as ain,
            tc.tile_pool(name="singles", bufs=1) as singles,
            tc.tile_pool(name="psT", bufs=2, space="PSUM") as psT_pool,
            tc.tile_pool(name="ps", bufs=6, space="PSUM") as ps_pool,
        ):
            # ---- input DMAs ----
            a_sb = []
            for k in range(KT):
                a_t = ain.tile([128, SLAB], bf16)
                nc.sync.dma_start(out=a_t, in_=at_r[k])
                a_sb.append(a_t)

            w_sb = singles.tile([128, KT, NOP], bf16)
            nc.sync.dma_start(out=w_sb, in_=wt_r)

            z_sb = {}
            for g in range(NG):
                for k in range(KT):
                    z_t = zin.tile([128, GW], bf16)
                    nc.sync.dma_start(
                        out=z_t, in_=zt_r[k][:, g * GW:(g + 1) * GW]
                    )
                    z_sb[(g, k)] = z_t

            partials = singles.tile([128, MT, NJ], f32)
            es_sb = singles.tile([128, MT], f32)
            tm_sb = singles.tile([128, MT, NOP], f32)

            # ---- class-segment sums: tm[:, m, c] = A_m @ W.T ----
            for m in range(MT):
                pst = psT_pool.tile([128, NOP], f32)
                for k in range(KT):
                    nc.tensor.matmul(
                        pst,
                        a_sb[k][:, m * 128:(m + 1) * 128],
                        w_sb[:, k, :],
                        start=(k == 0),
                        stop=(k == KT - 1),
                    )
                nc.vector.tensor_copy(tm_sb[:, m, :], pst)

            # ---- main similarity slab + fused exp row-sums ----
            for g in range(NG):
                for m in range(MT):
                    ps_t = [
                        ps_pool.tile([128, JW], f32, name="ps_t", tag="ps_t")
                        for _ in range(GJ)
                    ]
                    for k in range(KT):
                        lhsT = a_sb[k][:, m * 128:(m + 1) * 128]
                        for jj in range(GJ):
                            nc.tensor.matmul(
                                ps_t[jj],
                                lhsT,
                                z_sb[(g, k)][:, jj * JW:(jj + 1) * JW],
                                start=(k == 0),
                                stop=(k == KT - 1),
                            )
                    for jj in range(GJ):
                        j = g * GJ + jj
                        nc.scalar.activation(
                            out=ps_t[jj],
                            in_=ps_t[jj],
                            func=Exp,
                            scale=TEMP_INV,
                            accum_out=partials[:, m, j:j + 1],
                        )

            # ---- finish: sum the NJ exp partials per anchor, DMA out ----
            nc.vector.tensor_reduce(
                out=es_sb,
                in_=partials,
                axis=mybir.AxisListType.X,
                op=mybir.AluOpType.add,
            )
            nc.sync.dma_start(out=es, in_=es_sb)
            nc.sync.dma_start(out=tm, in_=tm_sb)

    nc.compile()
    return nc


def _get_nc():
    if "nc" not in _CACHE:
        _CACHE["nc"] = _build_nc()
    return _CACHE["nc"]


def kernel(x, op_ids, n_op):
    global LAST_RESULT
    from concourse.bass_utils import run_bass_kernel_spmd

    x = np.asarray(x, dtype=np.float32).reshape(-1, D)
    op_ids = np.asarray(op_ids).reshape(-1).astype(np.int64)
    n_op_i = int(np.asarray(n_op))

    # ---- host prep: normalize, cast bf16, class sums, diagonal ----
    norms = np.sqrt((x.astype(np.float64) ** 2).sum(axis=1))
    norms = np.maximum(norms, EPS).astype(np.float32)
    z = x / norms[:, None]
    z_bf16 = z.astype(BF16)
    zf = z_bf16.astype(np.float32)

    onehot = np.zeros((N, NOP), np.float32)
    onehot[np.arange(N), op_ids] = 1.0
    W = onehot.T @ zf                               # [NOP, D]

    zt_np = np.ascontiguousarray(z_bf16.T)          # [D, N] bf16
    wt_np = np.ascontiguousarray(W.T.astype(BF16))  # [D, NOP] bf16
    ssq = (zf.astype(np.float64) ** 2).sum(axis=1)  # = sim_raw[i, i]

    in_maps = [
        {
            "zt": zt_np,
            "at": np.ascontiguousarray(zt_np[:, c * SLAB:(c + 1) * SLAB]),
            "wt": wt_np,
        }
        for c in range(CORES)
    ]

    nc = _get_nc()
    res = run_bass_kernel_spmd(nc, in_maps, core_ids=list(range(CORES)))
    LAST_RESULT = res

    # ---- host post: stitch slabs, subtract diagonal, finish loss ----
    es_full = np.concatenate(
        [res.results[c]["es"].T.reshape(SLAB) for c in range(CORES)]
    ).astype(np.float64)
    tm_full = np.concatenate(
        [res.results[c]["tm"].transpose(1, 0, 2).reshape(SLAB, NOP)
         for c in range(CORES)]
    ).astype(np.float64)

    lse = np.log(es_full - np.exp(TEMP_INV * ssq))
    pos_sum = TEMP_INV * (tm_full[np.arange(N), op_ids] - ssq)
    counts = np.bincount(op_ids, minlength=n_op_i).astype(np.float64)
    pos_cnt = counts[op_ids] - 1.0

    loss_i = np.where(pos_cnt > 0, -pos_sum / np.maximum(pos_cnt, 1.0) + lse, 0.0)
    cls_sum = np.bincount(op_ids, weights=loss_i, minlength=n_op_i)
    cls_loss = np.where(counts > 0, cls_sum / np.maximum(counts, 1.0), 0.0)
    return np.float32(cls_loss.mean())


# revision 4
# speedup vs baseline: 1.7476x; 1.7476x over previous
"""Supervised-contrastive loss on 8 Trainium2 NeuronCores.

Math (reference):
    z = x / max(||x||, 1e-8)                  row-normalize
    sim = (z @ z.T) / TEMP                    [N, N]
    per-anchor: pos-mean over same-class (excl. self) and logsumexp over
    j != i, then per-class mean, then mean over classes.

Distribution: rows of z (anchors) are sharded 1024/core across 8 cores;
z is replicated.  Each core computes its [1024, 8192] slab of raw cosine
similarities (fp8-e4m3 inputs via DoubleRow matmuls, fp32 accumulate) and
produces two tiny reductions of it:
    es[i]    = sum_j exp(10 * sim8[i, j])             (incl. diagonal)
    tm[i, c] = sum_{j in class c} simb[i, j]          (incl. diagonal)
The class-segment sums are folded into a small bf16 GEMM: tm = A @ W.T
where W[c] = sum of z rows of class c (precomputed on host), so no
masking is needed on device.  The diagonal terms (sim[i,i] = ||z_q[i]||^2
in the matching quantization) are reconstructed exactly on host and
subtracted there.  The final logsumexp/segment-mean arithmetic over 8192
anchors is negligible host work.
"""

import numpy as np
import ml_dtypes

N = 8192          # anchors
D = 768           # feature dim
NOP = 64          # number of classes
CORES = 8
SLAB = N // CORES  # 1024 anchors per core
KT = D // 128      # 6 contraction tiles (bf16 T-phase)
KT8 = D // 256     # 3 double-row contraction tiles (fp8 main GEMM)
MT = SLAB // 128   # 8 anchor chunks of 128 (PSUM partition dim)
JW = 512           # matmul free width (one PSUM bank)
GW = 2048          # j-group width (one wide PSUM tile / DMA chunk)
NG = N // GW       # 4 groups
GJ = GW // JW      # 4 matmul slices per group
TEMP_INV = 10.0
EPS = 1e-8

BF16 = ml_dtypes.bfloat16
FP8 = ml_dtypes.float8_e4m3

_CACHE = {}
LAST_RESULT = None  # BassKernelResults of the most recent run (for profiling)


def _build_nc():
    from concourse import bacc
    import concourse.mybir as mybir
    import concourse.tile as tile

    bf16 = mybir.dt.bfloat16
    f8 = mybir.dt.float8e4
    f32 = mybir.dt.float32
    Exp = mybir.ActivationFunctionType.Exp
    DR = mybir.MatmulPerfMode.DoubleRow

    nc = bacc.Bacc(
        "TRN2", target_bir_lowering=False, debug=False, enable_asserts=False
    )
    zt8 = nc.dram_tensor("zt8", [D, N], f8, kind="ExternalInput").ap()
    at8 = nc.dram_tensor("at8", [D, SLAB], f8, kind="ExternalInput").ap()
    atb = nc.dram_tensor("atb", [D, SLAB], bf16, kind="ExternalInput").ap()
    wt = nc.dram_tensor("wt", [D, NOP], bf16, kind="ExternalInput").ap()
    es = nc.dram_tensor("es", [128, MT], f32, kind="ExternalOutput").ap()
    tm = nc.dram_tensor("tm", [128, MT, NOP], f32, kind="ExternalOutput").ap()

    # d = kk*256 + i*128 + p  (i = DoubleRow plane)
    zt8_r = zt8.rearrange("(kk i p) n -> kk p i n", i=2, p=128)  # [KT8,128,2,N]
    at8_r = at8.rearrange("(kk i p) n -> kk p i n", i=2, p=128)
    atb_r = atb.rearrange("(k p) n -> k p n", p=128)             # [KT,128,SLAB]
    wt_r = wt.rearrange("(k p) c -> p k c", p=128)               # [128,KT,NOP]

    with tile.TileContext(nc) as tc:
        with (
            tc.tile_pool(name="zin", bufs=KT8 * NG) as zin,
            tc.tile_pool(name="ain", bufs=KT8 + KT) as ain,
            tc.tile_pool(name="singles", bufs=1) as singles,
        ):
            # ---- input DMAs ----
            ab_sb = []
            for k in range(KT):
                ab_t = ain.tile([128, SLAB], bf16, name="ab_t", tag="ab_t")
                nc.sync.dma_start(out=ab_t, in_=atb_r[k])
                ab_sb.append(ab_t)

            w_sb = singles.tile([128, KT, NOP], bf16)
            nc.sync.dma_start(out=w_sb, in_=wt_r)

            a8_sb = []
            for kk in range(KT8):
                a8_t = ain.tile([128, 2, SLAB], f8, name="a8_t", tag="a8_t")
                nc.sync.dma_start(out=a8_t, in_=at8_r[kk])
                a8_sb.append(a8_t)

            z8_sb = {}
            for g in range(NG):
                for kk in range(KT8):
                    z8_t = zin.tile([128, 2, GW], f8, name="z8_t", tag="z8_t")
                    nc.sync.dma_start(
                        out=z8_t, in_=zt8_r[kk][:, :, g * GW:(g + 1) * GW]
                    )
                    z8_sb[(g, kk)] = z8_t

            partials = singles.tile([128, MT, NG], f32)
            es_sb = singles.tile([128, MT], f32)
            tm_sb = singles.tile([128, MT, NOP], f32)

            # ---- class-segment sums: tm[:, m, c] = A_m @ W.T (bf16) ----
            with tc.tile_pool(name="psT", bufs=2, space="PSUM") as psT_pool:
                for m in range(MT):
                    pst = psT_pool.tile([128, NOP], f32, name="pst", tag="pst")
                    for k in range(KT):
                        nc.tensor.matmul(
                            pst,
                            ab_sb[k][:, m * 128:(m + 1) * 128],
                            w_sb[:, k, :],
                            start=(k == 0),
                            stop=(k == KT - 1),
                        )
                    nc.vector.tensor_copy(tm_sb[:, m, :], pst)

            # ---- main similarity slab (fp8 DoubleRow) + fused exp sums ----
            with tc.tile_pool(name="ps", bufs=2, space="PSUM") as ps_pool:
                for g in range(NG):
                    for m in range(MT):
                        ps_t = ps_pool.tile([128, GW], f32, name="ps_t", tag="ps_t")
                        for kk in range(KT8):
                            lhsT = a8_sb[kk][:, :, m * 128:(m + 1) * 128]
                            for jj in range(GJ):
                                nc.tensor.matmul(
                                    ps_t[:, jj * JW:(jj + 1) * JW],
                                    lhsT,
                                    z8_sb[(g, kk)][:, :, jj * JW:(jj + 1) * JW],
                                    start=(kk == 0),
                                    stop=(kk == KT8 - 1),
                                    perf_mode=DR,
                                )
                        nc.scalar.activation(
                            out=ps_t,
                            in_=ps_t,
                            func=Exp,
                            scale=TEMP_INV,
                            accum_out=partials[:, m, g:g + 1],
                        )

            # ---- finish: sum the NG exp partials per anchor, DMA out ----
            nc.vector.tensor_reduce(
                out=es_sb,
                in_=partials,
                axis=mybir.AxisListType.X,
                op=mybir.AluOpType.add,
            )
            nc.sync.dma_start(out=es, in_=es_sb)
            nc.sync.dma_start(out=tm, in_=tm_sb)

    nc.compile()
    return nc


def _get_nc():
    if "nc" not in _CACHE:
        _CACHE["nc"] = _build_nc()
    return _CACHE["nc"]


def kernel(x, op_ids, n_op):
    global LAST_RESULT
    from concourse.bass_utils import run_bass_kernel_spmd

    x = np.asarray(x, dtype=np.float32).reshape(-1, D)
    op_ids = np.asarray(op_ids).reshape(-1).astype(np.int64)
    n_op_i = int(np.asarray(n_op))

    # ---- host prep: normalize, quantize, class sums, diagonals ----
    norms = np.sqrt((x.astype(np.float64) ** 2).sum(axis=1))
    norms = np.maximum(norms, EPS).astype(np.float32)
    z = x / norms[:, None]

    z8 = z.astype(FP8)
    z8f = z8.astype(np.float32)
    zb = z.astype(BF16)
    zbf = zb.astype(np.float32)

    onehot = np.zeros((N, NOP), np.float32)
    onehot[np.arange(N), op_ids] = 1.0
    W = onehot.T @ zbf                              # [NOP, D]

    zt8_np = np.ascontiguousarray(z8.T)             # [D, N] fp8
    atb_np = np.ascontiguousarray(zb.T)             # [D, N] bf16 (sliced below)
    wt_np = np.ascontiguousarray(W.T.astype(BF16))  # [D, NOP] bf16
    ssq8 = (z8f.astype(np.float64) ** 2).sum(axis=1)  # fp8 GEMM diagonal
    ssqb = (zbf.astype(np.float64) ** 2).sum(axis=1)  # bf16 T-path diagonal

    in_maps = [
        {
            "zt8": zt8_np,
            "at8": np.ascontiguousarray(zt8_np[:, c * SLAB:(c + 1) * SLAB]),
            "atb": np.ascontiguousarray(atb_np[:, c * SLAB:(c + 1) * SLAB]),
            "wt": wt_np,
        }
        for c in range(CORES)
    ]

    nc = _get_nc()
    res = run_bass_kernel_spmd(nc, in_maps, core_ids=list(range(CORES)))
    LAST_RESULT = res

    # ---- host post: stitch slabs, subtract diagonals, finish loss ----
    es_full = np.concatenate(
        [res.results[c]["es"].T.reshape(SLAB) for c in range(CORES)]
    ).astype(np.float64)
    tm_full = np.concatenate(
        [res.results[c]["tm"].transpose(1, 0, 2).reshape(SLAB, NOP)
         for c in range(CORES)]
    ).astype(np.float64)

    lse = np.log(es_full - np.exp(TEMP_INV * ssq8))
    pos_sum = TEMP_INV * (tm_full[np.arange(N), op_ids] - ssqb)
    counts = np.bincount(op_ids, minlength=n_op_i).astype(np.float64)
    pos_cnt = counts[op_ids] - 1.0

    loss_i = np.where(pos_cnt > 0, -pos_sum / np.maximum(pos_cnt, 1.0) + lse, 0.0)
    cls_sum = np.bincount(op_ids, weights=loss_i, minlength=n_op_i)
    cls_loss = np.where(counts > 0, cls_sum / np.maximum(counts, 1.0), 0.0)
    return np.float32(cls_loss.mean())
